# revision 4
# baseline (speedup 1.0000x reference)
"""DCNv2 block kernel for 8 Trainium2 NeuronCores.

Sharding: 8 cores = 4 batch samples x 2 row-halves (32 output rows each).

Per core pipeline (all on-device):
  1. Offset conv (3x3, 27 out ch) on PE from a host-padded channel-major slab.
  2. Transpose conv output to point-major, compute bilinear coords/weights on
     DVE (fp32, robust floor), fold sigmoid(mask) into the 4 corner weights.
     Validity handling is free: indices are clamped into a zero-bordered
     padded grid, and far-out-of-range points map to an all-zero row.
  3. Hardware indirect DMA (qPoolDynamic): one index per (position, tap)
     fetches a 1024-elem bf16 "quad row" = all 4 bilinear corners x 256ch
     from a host-prebuilt quad table in DRAM.
  4. Blend 4 corners with scalar_tensor_tensor (per-partition scalars).
  5. PE-transpose blended tiles to contraction-major, accumulate 18 matmuls
     (k-tap x c-chunk) into PSUM per 512-position superblock, DMA out fp32.
"""

import functools
import sys

import numpy as np

sys.path.insert(0, "/opt/trn_rl_repo")

import ml_dtypes  # noqa: E402

import concourse.bacc as bacc  # noqa: E402
import concourse.bass as bass  # noqa: E402
import concourse.mybir as mybir  # noqa: E402
import concourse.tile as tile  # noqa: E402

F32 = mybir.dt.float32
BF16 = mybir.dt.bfloat16
I32 = mybir.dt.int32
AF = mybir.ActivationFunctionType
OP = mybir.AluOpType

B, CIN, COUT, H, W, K = 4, 256, 256, 64, 64, 3
KK = K * K
ROWS = 32          # output rows per core
NPOS = ROWS * W    # 2048
NBLK = 16          # 2-row position blocks
NTAB = 66 * 66     # quad table rows


def build_nc(repeat: int = 1) -> bass.Bass:
    """repeat>1 re-runs the main gather/blend/matmul loop (timing only)."""
    from contextlib import ExitStack

    nc = bacc.Bacc("TRN2")
    qtab = nc.dram_tensor("qtab", [NTAB, 1024], BF16, kind="ExternalInput")
    xslab = nc.dram_tensor("xslab", [2, 128, 34, 66], F32, kind="ExternalInput")
    woff = nc.dram_tensor("woff", [128, 18 * 27], F32, kind="ExternalInput")
    offb = nc.dram_tensor("offb", [27, 1], F32, kind="ExternalInput")
    wmain = nc.dram_tensor("wmain", [128, 36 * 128], BF16, kind="ExternalInput")
    eyeb = nc.dram_tensor("eyeb", [128, 128], BF16, kind="ExternalInput")
    eyef = nc.dram_tensor("eyef", [27, 27], F32, kind="ExternalInput")
    byd = nc.dram_tensor("by", [128, 144], F32, kind="ExternalInput")
    bxd = nc.dram_tensor("bx", [128, 144], F32, kind="ExternalInput")
    y = nc.dram_tensor("y", [256, NPOS], F32, kind="ExternalOutput")

    with tile.TileContext(nc) as tc, ExitStack() as ctx:
        const = ctx.enter_context(tc.tile_pool(name="const", bufs=1))
        slabp = ctx.enter_context(tc.tile_pool(name="slab", bufs=1))
        cpool = ctx.enter_context(tc.tile_pool(name="coord", bufs=1))
        gpool = ctx.enter_context(tc.tile_pool(name="gath", bufs=3))
        spool = ctx.enter_context(tc.tile_pool(name="samp", bufs=2))
        stp = ctx.enter_context(tc.tile_pool(name="sT", bufs=2))
        outp = ctx.enter_context(tc.tile_pool(name="out", bufs=2))
        ptr = ctx.enter_context(tc.tile_pool(name="ptr", bufs=1, space="PSUM"))
        pconv = ctx.enter_context(tc.tile_pool(name="pconv", bufs=1, space="PSUM"))
        ptm = ctx.enter_context(tc.tile_pool(name="ptm", bufs=2, space="PSUM"))
        pmat = ctx.enter_context(tc.tile_pool(name="pmat", bufs=2, space="PSUM"))

        # ---- constants ----
        eyeb_t = const.tile([128, 128], BF16)
        nc.sync.dma_start(eyeb_t[:], eyeb[:])
        eyef_t = const.tile([27, 27], F32)
        nc.sync.dma_start(eyef_t[:], eyef[:])
        woff_t = const.tile([128, 18 * 27], F32)
        nc.sync.dma_start(woff_t[:], woff[:])
        offb_t = const.tile([27, 1], F32)
        nc.sync.dma_start(offb_t[:], offb[:])
        wmain_t = const.tile([128, 36, 128], BF16)
        nc.sync.dma_start(wmain_t[:], wmain[:].rearrange("p (a b) -> p a b", b=128))
        by_t = const.tile([128, 144], F32)
        nc.sync.dma_start(by_t[:], byd[:])
        bx_t = const.tile([128, 144], F32)
        nc.sync.dma_start(bx_t[:], bxd[:])

        # ---- offset conv ----
        xs = []
        for cc in range(2):
            t = slabp.tile([128, 34, 66], F32, tag=f"slab{cc}")
            nc.sync.dma_start(t[:], xslab[cc])
            xs.append(t)
        o_sb = cpool.tile([27, NPOS], F32)
        for p4 in range(4):
            ps = pconv.tile([27, 512], F32)
            n = 0
            for cc in range(2):
                for k in range(KK):
                    ki, kj = k // K, k % K
                    nc.tensor.matmul(
                        ps[:],
                        woff_t[:, (k * 2 + cc) * 27 : (k * 2 + cc + 1) * 27],
                        xs[cc][:, p4 * 8 + ki : p4 * 8 + ki + 8, kj : kj + 64],
                        start=(n == 0),
                        stop=(n == 17),
                    )
                    n += 1
            nc.scalar.activation(
                o_sb[:, p4 * 512 : (p4 + 1) * 512], ps[:], AF.Identity, bias=offb_t[:]
            )

        # ---- transpose offsets to point-major: OT [128, 16, 27] ----
        OT = cpool.tile([128, 16, 27], F32)
        for blk in range(NBLK):
            pT = ptr.tile([128, 27], F32, tag="pT27")
            nc.tensor.transpose(pT[:], o_sb[:, blk * 128 : (blk + 1) * 128], eyef_t[:])
            nc.scalar.activation(OT[:, blk, :], pT[:], AF.Copy)

        # ---- coords / weights / indices (fp32, [128,144] = (blk, tap)) ----
        DY = OT[:, :, 0:18:2]
        DX = OT[:, :, 1:18:2]
        MS = OT[:, :, 18:27]

        def ctile():
            ctile.n += 1
            return cpool.tile([128, 144], F32, tag=f"c{ctile.n}", name=f"c{ctile.n}")

        ctile.n = 0

        def floor_map(dsl, base_t, hi):
            """p = base + d; z = floor(p); w1 = frac; Z0 = padded-grid index:
            clamp(z+1, 0, hi) mapped to hi when z+1 < 0 (both corners OOB)."""
            p = ctile()
            nc.vector.tensor_tensor(p[:], dsl, base_t[:], OP.add)
            ci = cpool.tile([128, 144], I32, tag=f"i{ctile.n}", name=f"i{ctile.n}")
            nc.vector.tensor_copy(ci[:], p[:])
            cf = ctile()
            nc.vector.tensor_copy(cf[:], ci[:])
            gt = ctile()
            nc.vector.tensor_tensor(gt[:], cf[:], p[:], OP.is_gt)
            z = ctile()
            nc.vector.tensor_tensor(z[:], cf[:], gt[:], OP.subtract)
            w1 = ctile()
            nc.vector.tensor_tensor(w1[:], p[:], z[:], OP.subtract)
            t = ctile()
            nc.vector.tensor_scalar(t[:], z[:], 1.0, None, OP.add)
            tclamp = ctile()
            nc.vector.tensor_scalar(tclamp[:], t[:], 0.0, float(hi), OP.max, OP.min)
            lt = ctile()
            nc.vector.tensor_scalar(lt[:], t[:], 0.0, None, OP.is_lt)
            z0 = ctile()
            nc.vector.scalar_tensor_tensor(
                z0[:], lt[:], float(hi), tclamp[:], OP.mult, OP.add
            )
            return z0, w1

        Y0, wy1 = floor_map(DY, by_t, 65)
        X0, wx1 = floor_map(DX, bx_t, 65)

        msig = ctile()
        nc.scalar.activation(msig[:], MS, AF.Sigmoid)

        wy0 = ctile()
        nc.vector.tensor_scalar(wy0[:], wy1[:], -1.0, 1.0, OP.mult, OP.add)
        wx0 = ctile()
        nc.vector.tensor_scalar(wx0[:], wx1[:], -1.0, 1.0, OP.mult, OP.add)
        t0 = ctile()
        nc.vector.tensor_tensor(t0[:], wx0[:], msig[:], OP.mult)
        t1 = ctile()
        nc.vector.tensor_tensor(t1[:], wx1[:], msig[:], OP.mult)

        betas = cpool.tile([128, 4, 144], F32)
        nc.vector.tensor_tensor(betas[:, 0, :], wy0[:], t0[:], OP.mult)
        nc.vector.tensor_tensor(betas[:, 1, :], wy0[:], t1[:], OP.mult)
        nc.vector.tensor_tensor(betas[:, 2, :], wy1[:], t0[:], OP.mult)
        nc.vector.tensor_tensor(betas[:, 3, :], wy1[:], t1[:], OP.mult)

        # idx = Y0*66 + X0, exact in fp32, then to int32
        idxf = ctile()
        nc.vector.scalar_tensor_tensor(idxf[:], Y0[:], 66.0, X0[:], OP.mult, OP.add)
        IDX = cpool.tile([128, 144], I32)
        nc.vector.tensor_copy(IDX[:], idxf[:])

        # ---- main loop: gather / blend / transpose / matmul ----
        sT = None
        for blk in [b for _ in range(repeat) for b in range(NBLK)]:
            gq = gpool.tile([128, 9, 1024], BF16, tag="g")
            for k in range(KK):
                c = blk * 9 + k
                nc.gpsimd.indirect_dma_start(
                    out=gq[:, k, :],
                    out_offset=None,
                    in_=qtab[:],
                    in_offset=bass.IndirectOffsetOnAxis(ap=IDX[:, c : c + 1], axis=0),
                )
            s = spool.tile([128, 2304], BF16, tag="s")
            for k in range(KK):
                c = blk * 9 + k
                sk = s[:, k * 256 : (k + 1) * 256]
                nc.vector.tensor_scalar(
                    sk, gq[:, k, 0:256], betas[:, 0, c : c + 1], None, OP.mult
                )
                for n in range(1, 4):
                    nc.vector.scalar_tensor_tensor(
                        sk,
                        gq[:, k, n * 256 : (n + 1) * 256],
                        betas[:, n, c : c + 1],
                        sk,
                        OP.mult,
                        OP.add,
                    )
            if blk % 4 == 0:
                sT = stp.tile([128, 18, 512], BF16, tag="sT")
            col = (blk % 4) * 128
            for t2 in range(18):
                if t2 % 4 == 0:
                    pt2 = ptm.tile([128, 512], BF16, tag="pt2")
                nc.tensor.transpose(
                    pt2[:, (t2 % 4) * 128 : (t2 % 4 + 1) * 128],
                    s[:, t2 * 128 : (t2 + 1) * 128],
                    eyeb_t[:],
                )
                if t2 % 4 == 3 or t2 == 17:
                    j0 = (t2 // 4) * 4
                    cnt = t2 % 4 + 1
                    nc.scalar.activation(
                        sT[:, j0 : j0 + cnt, col : col + 128],
                        pt2[:, : cnt * 128],
                        AF.Copy,
                    )
            if blk % 4 == 3:
                sb = blk // 4
                for half in range(2):
                    pm = pmat.tile([128, 512], F32, tag="pm")
                    for t2 in range(18):
                        nc.tensor.matmul(
                            pm[:],
                            wmain_t[:, t2 * 2 + half, :],
                            sT[:, t2, :],
                            start=(t2 == 0),
                            stop=(t2 == 17),
                        )
                    ob = outp.tile([128, 512], F32, tag="ob")
                    nc.vector.tensor_copy(ob[:], pm[:])
                    nc.sync.dma_start(
                        y[half * 128 : (half + 1) * 128, sb * 512 : (sb + 1) * 512],
                        ob[:],
                    )
    nc.compile()
    return nc


@functools.lru_cache(maxsize=1)
def _get_nc():
    return build_nc()


@functools.lru_cache(maxsize=1)
def _static_inputs():
    """Per-core input tensors that do not depend on runtime data values."""
    eyeb = np.eye(128, dtype=ml_dtypes.bfloat16)
    eyef = np.eye(27, dtype=np.float32)
    per_half = []
    k = np.arange(9)
    ki, kj = k // K, k % K
    lane = np.arange(128)
    blk = np.arange(16)
    for half in range(2):
        r0 = half * ROWS
        row = r0 + 2 * blk[None, :, None] + (lane[:, None, None] // 64)
        col = lane[:, None, None] % 64 + np.zeros((1, 16, 1), np.int64)
        by = (row - 1 + ki[None, None, :]).astype(np.float32).reshape(128, 144)
        bx = (col - 1 + kj[None, None, :]).astype(np.float32).reshape(128, 144)
        per_half.append((by, bx))
    return eyeb, eyef, per_half


def _prep_weights(offset_w, offset_b, dcn_w):
    # woff[c, (k,cc), o] = offset_w[o, cc*128+c, ki, kj]
    ow = offset_w.reshape(27, 2, 128, 3, 3)
    woff = np.ascontiguousarray(
        np.transpose(ow, (2, 3, 4, 1, 0)).reshape(128, 9 * 2 * 27)
    ).astype(np.float32)
    offb = offset_b.reshape(27, 1).astype(np.float32)
    # wmain[c, (k,cc,half), o] = dcn_w[half*128+o, cc*128+c, ki, kj]
    dw = dcn_w.reshape(2, 128, 2, 128, 3, 3)
    wmain = np.ascontiguousarray(
        np.transpose(dw, (3, 4, 5, 2, 0, 1)).reshape(128, 36 * 128)
    ).astype(ml_dtypes.bfloat16)
    return woff, offb, wmain


def _build_qtab(xsamp):
    """Quad table [4356, 1024] bf16: row (y*66+x) = corners
    [xp[y,x,:], xp[y,x+1,:], xp[y+1,x,:], xp[y+1,x+1,:]] of the zero-padded
    channel-last image xp[67,67,256] (image at [1:65, 1:65])."""
    xp = np.zeros((67, 67, 256), dtype=ml_dtypes.bfloat16)
    xp[1:65, 1:65] = xsamp.transpose(1, 2, 0)
    q = np.empty((66, 66, 4, 256), dtype=ml_dtypes.bfloat16)
    q[:, :, 0] = xp[:66, :66]
    q[:, :, 1] = xp[:66, 1:67]
    q[:, :, 2] = xp[1:67, :66]
    q[:, :, 3] = xp[1:67, 1:67]
    return q.reshape(NTAB, 1024)


def make_in_maps(x, offset_w, offset_b, dcn_w):
    eyeb, eyef, per_half = _static_inputs()
    woff, offb, wmain = _prep_weights(
        np.asarray(offset_w), np.asarray(offset_b), np.asarray(dcn_w)
    )
    x = np.asarray(x, dtype=np.float32)
    qtabs = [_build_qtab(x[b]) for b in range(B)]
    in_maps = []
    for core in range(8):
        b, half = core // 2, core % 2
        r0 = half * ROWS
        xsamp = x[b]
        xp = np.zeros((2, 128, 34, 66), np.float32)
        lo, hi = r0 - 1, r0 + 33
        slo, shi = max(lo, 0), min(hi, H)
        xp[:, :, (slo - lo) : (slo - lo) + (shi - slo), 1:65] = xsamp.reshape(
            2, 128, H, W
        )[:, :, slo:shi, :]
        by, bx = per_half[half]
        in_maps.append(
            {
                "qtab": qtabs[b],
                "xslab": xp,
                "woff": woff,
                "offb": offb,
                "wmain": wmain,
                "eyeb": eyeb,
                "eyef": eyef,
                "by": by,
                "bx": bx,
            }
        )
    return in_maps


def kernel(x, offset_w, offset_b, dcn_w):
    from concourse.bass_utils import run_bass_kernel_spmd

    nc = _get_nc()
    in_maps = make_in_maps(x, offset_w, offset_b, dcn_w)
    out = np.zeros((B, COUT, H, W), np.float32)
    res = run_bass_kernel_spmd(nc, in_maps, core_ids=list(range(8)))
    for core in range(8):
        b, half = core // 2, core % 2
        r0 = half * ROWS
        out[b, :, r0 : r0 + ROWS, :] = np.asarray(res.results[core]["y"]).reshape(
            COUT, ROWS, W
        )
    return out
